# revision 18
# baseline (speedup 1.0000x reference)
"""TRN2 Bass kernel for nn_Attender:
    weights[b, s] = sum_d (state @ W.T + bias)[b, d] * enc[s, b, d]
with enc [S=2048, B=16, D=2048], state [B, D], W [D, D], bias [D], out [B, S].

Sharding (8 NeuronCores): the contraction dim D is split into 8 slices of 256,
one per core. Each core computes alteredT[d_k, b] = (W[d_k, :] @ state.T +
bias[d_k]) — needing only a 256-row slice of W — and the partial score
partial_k[b, s] = sum_{d in d_k} altered[b, d] * enc[s, b, d]. The host sums
the 8 partials (a pure reduction un-shard); no cross-device communication.

Device layout (host-pretransposed so every DMA is partition-contiguous):
  enc  [2, 128, 16*S]  per-core enc slice, d on partitions (fp16):
                       [chunk c, partition p, (batch b, s)]
  wp   [128, 16*256]   wp[p, i*256+d] = W[k*256+d, i*128+p]   (lhsT tiles)
  sp   [128, 16*16]    sp[p, i*16+b]  = state[b, i*128+p]
  bk   [128, 2]        bk[p, c]       = bias[k*256 + c*128 + p]  (fp32)

The kernel is HBM/DMA-bound: ~18 MB per core (16.8 MB fp16 enc + 1.1 MB
constants + outputs) streams at the per-core DMA governor's sustained limit
(~390-410 B/ns; bursts ~420 then a clamp repays the average — measured), so
the DMA phase is ~45 us and everything else hides under it, plus ~11 us of
fixed NEFF entry/exit overhead. Structure (all measured to matter):
  - Constants ride the Act HWDGE queue, NOT the gpsimd SWDGE: a concurrent
    SWDGE stream costs ~20% of enc bandwidth (315 vs 390 B/ns measured).
  - enc streams on the sync (SP) HWDGE queue as tilesets of [4,4,4,2,1]
    batches plus a final batch split into s-pieces [1024,512,512], so the
    post-stream PE tail is one 512-col matmul pair (~0.5 us). Both d-chunks
    of a tileset ride one DMA (tile free layout (c, b_local, s), 16 KB
    contiguous runs per partition).
  - Output DMAs ride the Act queue (strided out descriptors interleaved into
    the sync queue's enc stream cost ~8% bandwidth — measured). The last
    group ships [4, 1536] early + [4, 512] after the final drain.
  - Main contraction on the PE: lhsT = alteredT[d_chunk, b] (M=1), rhs =
    encT[d_chunk, s] (N=512), accumulating the 2 d-chunks in PSUM. The 4
    batches of a group land in one PSUM bank at partitions {0,32,64,96} via
    tile_position col-tiling. Loops run s-tile-outer so each [128, 512]
    drain (DVE/ACT alternating) fires as soon as its s-tile's 4 batches are
    done, overlapping the remaining matmuls. PE throughput (~215 ns per
    512-col fp16 matmul, pipelined) has 2x slack vs the stream.

Precision: enc/W/state/altered in fp16, fp32 PSUM accumulate. Measured error:
max|err| = 3.7e-4 * max|ref| = 1.5e-3 * rms(ref); resid_var 1.6e-7 — pure
input-rounding error (matches an exact numpy fp16 simulation).
"""

import os
from contextlib import ExitStack

import numpy as np

import concourse.bacc as bacc
import concourse.tile as tile
import concourse.mybir as mybir
from concourse.bass_utils import run_bass_kernel_spmd

S, B, D = 2048, 16, 2048
NCORES = 8
DK = D // NCORES  # 256 contraction elems per core
NCH = DK // 128  # 2 partition chunks
BG = 4  # batches per psum group
NG = B // BG  # 4 groups
ST = 512  # s-tile (one PSUM bank)
NST = S // ST  # 4 s-tiles

# enc streaming plan: full-batch tilesets for b0..14, then batch 15 split
# into s-pieces so the post-stream compute tail is small.
TS_FULL = [4, 4, 4, 2, 1]  # batches 0..14
# s-ranges of batch 15. Pieces below 512 are counterproductive: their small
# DMA descriptors (<1 KB) collapse to ~25 B/ns in the governor's clamped
# end-of-stream regime (measured: [.., 256, 256] cost ~8 us vs [.., 512]).
PIECES = [1024, 512, 512]
assert sum(TS_FULL) == B - 1
assert sum(PIECES) == S

MODE = os.environ.get("BASS_KERNEL_MODE", "fp16x1")

F32 = mybir.dt.float32
F16 = mybir.dt.float16

_CACHE = {}

LAST_RESULTS = None


def _build(mode):
    assert mode == "fp16x1", f"only fp16x1 supported, got {mode}"
    nc = bacc.Bacc("TRN2", target_bir_lowering=False, debug=False, num_devices=NCORES)

    ts_start = [sum(TS_FULL[:i]) for i in range(len(TS_FULL))]
    b2ts = {}
    for t, (sz, st0) in enumerate(zip(TS_FULL, ts_start)):
        for j in range(sz):
            b2ts[st0 + j] = (t, j)
    p_start = [sum(PIECES[:i]) for i in range(len(PIECES))]

    ENC = nc.dram_tensor("enc", [NCH, 128, B * S], F16, kind="ExternalInput").ap()
    WP = nc.dram_tensor("wp", [128, 16 * DK], F16, kind="ExternalInput").ap()
    SP = nc.dram_tensor("sp", [128, 16 * B], F16, kind="ExternalInput").ap()
    BK = nc.dram_tensor("bk", [128, NCH], F32, kind="ExternalInput").ap()
    OUT = nc.dram_tensor("out", [B, S], F32, kind="ExternalOutput").ap()

    with tile.TileContext(nc) as tc, ExitStack() as ctx:
        cpool = ctx.enter_context(tc.tile_pool(name="const", bufs=1))
        epool = ctx.enter_context(tc.tile_pool(name="enc", bufs=1))
        opool = ctx.enter_context(tc.tile_pool(name="outp", bufs=2))
        apsum = ctx.enter_context(tc.tile_pool(name="apsum", bufs=2, space="PSUM"))
        mpsum = ctx.enter_context(tc.tile_pool(name="mpsum", bufs=6, space="PSUM"))

        # Constants on the Act HWDGE queue: off the enc stream's SP queue, and
        # NOT on the gpsimd SWDGE.
        wp_t = cpool.tile([128, 16 * DK], F16, tag="wp")
        nc.scalar.dma_start(wp_t[:], WP[:])
        sp_t = cpool.tile([128, 16 * B], F16, tag="sp")
        nc.scalar.dma_start(sp_t[:], SP[:])
        bk_t = cpool.tile([128, NCH], F32, tag="bk")
        nc.scalar.dma_start(bk_t[:], BK[:])

        # enc tilesets stream on the sync (SP) HWDGE queue. Both d-chunks ride
        # one DMA: tile free layout is (c, b_local, s).
        tsets = {}
        for t, (sz, st0) in enumerate(zip(TS_FULL, ts_start)):
            et = epool.tile([128, NCH * sz * S], F16, tag=f"enct{t}", name=f"e_{t}")
            nc.sync.dma_start(
                et[:].rearrange("p (c f) -> p c f", c=NCH),
                ENC[:, :, st0 * S : (st0 + sz) * S].rearrange("c p f -> p c f"),
            )
            tsets[t] = et
        ptiles = []
        for j, (plen, s0) in enumerate(zip(PIECES, p_start)):
            pt = epool.tile([128, NCH * plen], F16, tag=f"encp{j}", name=f"p_{j}")
            nc.sync.dma_start(
                pt[:].rearrange("p (c f) -> p c f", c=NCH),
                ENC[:, :, (B - 1) * S + s0 : (B - 1) * S + s0 + plen].rearrange(
                    "c p f -> p c f"
                ),
            )
            ptiles.append(pt)

        # alteredT[d, b] = sum_i W[d, i] * state[b, i] + bias[d], d on partitions.
        amats = []  # amats[c] = fp16 lhsT tile for chunk c
        for c in range(NCH):
            aps = apsum.tile([128, B], F32, tag="aps")
            for i in range(16):
                nc.tensor.matmul(
                    aps[:],
                    wp_t[:, i * DK + c * 128 : i * DK + (c + 1) * 128],
                    sp_t[:, i * B : (i + 1) * B],
                    start=(i == 0),
                    stop=(i == 15),
                )
            altf = cpool.tile([128, B], F32, tag=f"altf{c}")
            nc.vector.tensor_scalar_add(altf[:], aps[:], bk_t[:, c : c + 1])
            af = cpool.tile([128, B], F16, tag=f"af{c}")
            nc.vector.tensor_copy(af[:], altf[:])
            amats.append(af)

        def rhs_slice(b, st, c):
            """SBUF rhs [128, ST] for batch b < 15, s-tile st, chunk c."""
            t, bloc = b2ts[b]
            sz = TS_FULL[t]
            off = (c * sz + bloc) * S + st * ST
            return tsets[t][:, off : off + ST]

        out_r = OUT.rearrange("(g bi) s -> g bi s", bi=BG)
        for g in range(NG):
            last_g = g == NG - 1
            pts = [
                mpsum.tile([128, ST], F32, tag="mm", name=f"pt_{g}_{st}")
                for st in range(NST)
            ]
            outg = opool.tile([128, S], F32, tag="outg", name=f"outg_{g}")
            outg_r = outg[:].rearrange("(bi r) s -> bi r s", bi=BG)[:, 0]

            def mm_b15(st, sel=None):
                """b15's s-tile st matmuls, one pair per overlapping piece."""
                s_lo, s_hi = st * ST, (st + 1) * ST
                for j, (plen, s0) in enumerate(zip(PIECES, p_start)):
                    if sel is not None and j not in sel:
                        continue
                    lo, hi = max(s_lo, s0), min(s_hi, s0 + plen)
                    if lo >= hi:
                        continue
                    for c in range(NCH):
                        nc.tensor.matmul(
                            pts[st][96:97, lo - s_lo : hi - s_lo],
                            amats[c][:, B - 1 : B],
                            ptiles[j][:, (c * plen + lo - s0) : (c * plen + hi - s0)],
                            start=(c == 0),
                            stop=(c == NCH - 1),
                            tile_position=(0, 96),
                        )

            # s-tile-outer: each s-tile's PSUM bank completes after its 4
            # batches (8 matmuls), so its drain overlaps later matmuls.
            for st in range(NST):
                last_st = last_g and st == NST - 1
                for bi in range(BG):
                    b = g * BG + bi
                    if last_g and bi == BG - 1:
                        mm_b15(st)
                        continue
                    for c in range(NCH):
                        nc.tensor.matmul(
                            pts[st][32 * bi : 32 * bi + 1, :],
                            amats[c][:, b : b + 1],
                            rhs_slice(b, st, c),
                            start=(c == 0),
                            stop=(c == NCH - 1),
                            tile_position=(0, 32 * bi),
                        )
                dst = outg[:, st * ST : (st + 1) * ST]
                if last_st:
                    # Final s-tile: column-split drain runs on DVE and ACT in
                    # parallel (~0.35 us each vs 0.69 serial; engine copy cost
                    # scales with the free dim).
                    nc.vector.tensor_copy(
                        outg[:, st * ST : st * ST + ST // 2], pts[st][:, : ST // 2]
                    )
                    nc.scalar.copy(
                        outg[:, st * ST + ST // 2 : (st + 1) * ST],
                        pts[st][:, ST // 2 :],
                    )
                elif st % 2 == 0:
                    nc.vector.tensor_copy(dst, pts[st][:])
                else:
                    nc.scalar.copy(dst, pts[st][:])
                if last_g and st == NST - 2:
                    # st0-2 drained: ship [4, 1536] now so the final DMA after
                    # the last piece is only [4, 512]. This rides the SYNC
                    # queue: by issue time (~55 us) the enc stream is done (its
                    # descriptors all precede this in the queue), and keeping
                    # it off ACT lets the st3 drain start there immediately.
                    nc.sync.dma_start(
                        out_r[g][:, : (NST - 1) * ST], outg_r[:, : (NST - 1) * ST]
                    )
            if last_g:
                nc.scalar.dma_start(
                    out_r[g][:, (NST - 1) * ST :], outg_r[:, (NST - 1) * ST :]
                )
            else:
                # Ship the group's [4, S] rows (batch bi at partition 32*bi)
                # on the Act queue; enc keeps streaming on the sync queue.
                nc.scalar.dma_start(out_r[g], outg_r)

    nc.compile()
    return nc


def _prep_inputs(encoder_outputs, state, W, b):
    """Build the 8 per-core input maps (heavy layout work on host)."""
    f16 = np.float16
    in_maps = []
    # [S, B, D] -> [B, D, S] once
    encT = np.ascontiguousarray(encoder_outputs.transpose(1, 2, 0))
    spk = np.ascontiguousarray(
        state.T.reshape(16, 128, B).transpose(1, 0, 2).reshape(128, 16 * B)
    ).astype(f16)
    for k in range(NCORES):
        d0 = k * DK
        e = encT[:, d0 : d0 + DK, :]  # [B, DK, S]
        # -> [c, p, B, S]
        e = np.ascontiguousarray(e.reshape(B, NCH, 128, S).transpose(1, 2, 0, 3))
        enc_k = np.ascontiguousarray(e.astype(f16).reshape(NCH, 128, B * S))
        wp = np.ascontiguousarray(
            W[d0 : d0 + DK, :].T.reshape(16, 128, DK).transpose(1, 0, 2).reshape(128, 16 * DK)
        ).astype(f16)
        bk = np.ascontiguousarray(b[d0 : d0 + DK].reshape(NCH, 128).T)
        in_maps.append({"enc": enc_k, "wp": wp, "sp": spk, "bk": bk})
    return in_maps


def kernel(encoder_outputs, state, W, b):
    global LAST_RESULTS
    mode = MODE
    if mode not in _CACHE:
        _CACHE[mode] = _build(mode)
    nc = _CACHE[mode]
    in_maps = _prep_inputs(
        np.asarray(encoder_outputs, dtype=np.float32),
        np.asarray(state, dtype=np.float32),
        np.asarray(W, dtype=np.float32),
        np.asarray(b, dtype=np.float32),
    )
    res = run_bass_kernel_spmd(nc, in_maps, core_ids=list(range(NCORES)))
    LAST_RESULTS = res
    acc = np.zeros((B, S), dtype=np.float64)
    for k in range(NCORES):
        acc += res.results[k]["out"].astype(np.float64)
    return acc.astype(np.float32)


# revision 19
# speedup vs baseline: 1.0822x; 1.0822x over previous
"""TRN2 Bass kernel for nn_Attender:
    weights[b, s] = sum_d (state @ W.T + bias)[b, d] * enc[s, b, d]
with enc [S=2048, B=16, D=2048], state [B, D], W [D, D], bias [D], out [B, S].

Sharding (8 NeuronCores): the contraction dim D is split into 8 slices of 256,
one per core. Each core computes alteredT[d_k, b] = (W[d_k, :] @ state.T +
bias[d_k]) — needing only a 256-row slice of W — and the partial score
partial_k[b, s] = sum_{d in d_k} altered[b, d] * enc[s, b, d]. The host sums
the 8 partials (a pure reduction un-shard); no cross-device communication.

Device layout (host-pretransposed so every DMA is partition-contiguous):
  enc  [2, 128, 16*S]  per-core enc slice, d on partitions (fp16):
                       [chunk c, partition p, (batch b, s)]
  wp   [128, 16*256]   wp[p, i*256+d] = W[k*256+d, i*128+p]   (lhsT tiles)
  sp   [128, 16*16]    sp[p, i*16+b]  = state[b, i*128+p]
  bk   [128, 2]        bk[p, c]       = bias[k*256 + c*128 + p]  (fp32)

The kernel is HBM/DMA-bound: ~18 MB per core (16.8 MB fp16 enc + 1.1 MB
constants + outputs) streams at the per-core DMA governor's sustained limit
(~390-410 B/ns; bursts ~420 then a clamp repays the average — measured), so
the DMA phase is ~45 us and everything else hides under it, plus ~11 us of
fixed NEFF entry/exit overhead. Structure (all measured to matter):
  - Constants ride the Act HWDGE queue, NOT the gpsimd SWDGE: a concurrent
    SWDGE stream costs ~20% of enc bandwidth (315 vs 390 B/ns measured).
  - enc streams on the sync (SP) HWDGE queue as tilesets of [4,4,4,2,1]
    batches plus a final batch split into s-pieces [1024,512,512], so the
    post-stream PE tail is one 512-col matmul pair (~0.5 us). Both d-chunks
    of a tileset ride one DMA (tile free layout (c, b_local, s), 16 KB
    contiguous runs per partition).
  - Output DMAs ride the Act queue (strided out descriptors interleaved into
    the sync queue's enc stream cost ~8% bandwidth — measured). The last
    group ships [4, 1536] early + [4, 512] after the final drain.
  - Main contraction on the PE: lhsT = alteredT[d_chunk, b] (M=1), rhs =
    encT[d_chunk, s] (N=512), accumulating the 2 d-chunks in PSUM. The 4
    batches of a group land in one PSUM bank at partitions {0,32,64,96} via
    tile_position col-tiling. Loops run s-tile-outer so each [128, 512]
    drain (DVE/ACT alternating) fires as soon as its s-tile's 4 batches are
    done, overlapping the remaining matmuls. PE throughput (~215 ns per
    512-col fp16 matmul, pipelined) has 2x slack vs the stream.

Precision: enc/W/state/altered in fp16, fp32 PSUM accumulate. Measured error:
max|err| = 3.7e-4 * max|ref| = 1.5e-3 * rms(ref); resid_var 1.6e-7 — pure
input-rounding error (matches an exact numpy fp16 simulation).
"""

import os
from contextlib import ExitStack

import numpy as np

import concourse.bacc as bacc
import concourse.tile as tile
import concourse.mybir as mybir
from concourse.bass_utils import run_bass_kernel_spmd

S, B, D = 2048, 16, 2048
NCORES = 8
DK = D // NCORES  # 256 contraction elems per core
NCH = DK // 128  # 2 partition chunks
BG = 4  # batches per psum group
NG = B // BG  # 4 groups
ST = 512  # s-tile (one PSUM bank)
NST = S // ST  # 4 s-tiles

# enc streaming plan: full-batch tilesets for b0..14, then batch 15 split
# into s-pieces so the post-stream compute tail is small.
TS_FULL = [4, 4, 4, 2, 1]  # batches 0..14
# s-ranges of batch 15. Pieces below 512 are counterproductive: their small
# DMA descriptors (<1 KB) collapse to ~25 B/ns in the governor's clamped
# end-of-stream regime (measured: [.., 256, 256] cost ~8 us vs [.., 512]).
PIECES = [1024, 512, 512]
assert sum(TS_FULL) == B - 1
assert sum(PIECES) == S

MODE = os.environ.get("BASS_KERNEL_MODE", "fp16x1")

F32 = mybir.dt.float32
F16 = mybir.dt.float16

_CACHE = {}

LAST_RESULTS = None


def _build(mode):
    assert mode == "fp16x1", f"only fp16x1 supported, got {mode}"
    nc = bacc.Bacc("TRN2", target_bir_lowering=False, debug=False, num_devices=NCORES)

    ts_start = [sum(TS_FULL[:i]) for i in range(len(TS_FULL))]
    b2ts = {}
    for t, (sz, st0) in enumerate(zip(TS_FULL, ts_start)):
        for j in range(sz):
            b2ts[st0 + j] = (t, j)
    p_start = [sum(PIECES[:i]) for i in range(len(PIECES))]

    ENC = nc.dram_tensor("enc", [NCH, 128, B * S], F16, kind="ExternalInput").ap()
    WP = nc.dram_tensor("wp", [128, 16 * DK], F16, kind="ExternalInput").ap()
    SP = nc.dram_tensor("sp", [128, 16 * B], F16, kind="ExternalInput").ap()
    BK = nc.dram_tensor("bk", [128, NCH], F32, kind="ExternalInput").ap()
    OUT = nc.dram_tensor("out", [B, S], F32, kind="ExternalOutput").ap()

    with tile.TileContext(nc) as tc, ExitStack() as ctx:
        cpool = ctx.enter_context(tc.tile_pool(name="const", bufs=1))
        epool = ctx.enter_context(tc.tile_pool(name="enc", bufs=1))
        opool = ctx.enter_context(tc.tile_pool(name="outp", bufs=2))
        apsum = ctx.enter_context(tc.tile_pool(name="apsum", bufs=2, space="PSUM"))
        mpsum = ctx.enter_context(tc.tile_pool(name="mpsum", bufs=6, space="PSUM"))

        # Constants on the Act HWDGE queue: off the enc stream's SP queue, and
        # NOT on the gpsimd SWDGE.
        wp_t = cpool.tile([128, 16 * DK], F16, tag="wp")
        nc.scalar.dma_start(wp_t[:], WP[:])
        sp_t = cpool.tile([128, 16 * B], F16, tag="sp")
        nc.scalar.dma_start(sp_t[:], SP[:])
        bk_t = cpool.tile([128, NCH], F32, tag="bk")
        nc.scalar.dma_start(bk_t[:], BK[:])

        # enc tilesets stream on the sync (SP) HWDGE queue. Both d-chunks ride
        # one DMA: tile free layout is (c, b_local, s).
        tsets = {}
        for t, (sz, st0) in enumerate(zip(TS_FULL, ts_start)):
            et = epool.tile([128, NCH * sz * S], F16, tag=f"enct{t}", name=f"e_{t}")
            nc.sync.dma_start(
                et[:].rearrange("p (c f) -> p c f", c=NCH),
                ENC[:, :, st0 * S : (st0 + sz) * S].rearrange("c p f -> p c f"),
            )
            tsets[t] = et
        ptiles = []
        for j, (plen, s0) in enumerate(zip(PIECES, p_start)):
            pt = epool.tile([128, NCH * plen], F16, tag=f"encp{j}", name=f"p_{j}")
            nc.sync.dma_start(
                pt[:].rearrange("p (c f) -> p c f", c=NCH),
                ENC[:, :, (B - 1) * S + s0 : (B - 1) * S + s0 + plen].rearrange(
                    "c p f -> p c f"
                ),
            )
            ptiles.append(pt)

        # alteredT[d, b] = sum_i W[d, i] * state[b, i] + bias[d], d on partitions.
        amats = []  # amats[c] = fp16 lhsT tile for chunk c
        for c in range(NCH):
            aps = apsum.tile([128, B], F32, tag="aps")
            for i in range(16):
                nc.tensor.matmul(
                    aps[:],
                    wp_t[:, i * DK + c * 128 : i * DK + (c + 1) * 128],
                    sp_t[:, i * B : (i + 1) * B],
                    start=(i == 0),
                    stop=(i == 15),
                )
            altf = cpool.tile([128, B], F32, tag=f"altf{c}")
            nc.vector.tensor_scalar_add(altf[:], aps[:], bk_t[:, c : c + 1])
            af = cpool.tile([128, B], F16, tag=f"af{c}")
            nc.vector.tensor_copy(af[:], altf[:])
            amats.append(af)

        def rhs_slice(b, st, c):
            """SBUF rhs [128, ST] for batch b < 15, s-tile st, chunk c."""
            t, bloc = b2ts[b]
            sz = TS_FULL[t]
            off = (c * sz + bloc) * S + st * ST
            return tsets[t][:, off : off + ST]

        out_r = OUT.rearrange("(g bi) s -> g bi s", bi=BG)
        for g in range(NG):
            last_g = g == NG - 1
            pts = [
                mpsum.tile([128, ST], F32, tag="mm", name=f"pt_{g}_{st}")
                for st in range(NST)
            ]
            outg = opool.tile([128, S], F32, tag="outg", name=f"outg_{g}")
            outg_r = outg[:].rearrange("(bi r) s -> bi r s", bi=BG)[:, 0]

            def mm_b15(st, sel=None):
                """b15's s-tile st matmuls, one pair per overlapping piece."""
                s_lo, s_hi = st * ST, (st + 1) * ST
                for j, (plen, s0) in enumerate(zip(PIECES, p_start)):
                    if sel is not None and j not in sel:
                        continue
                    lo, hi = max(s_lo, s0), min(s_hi, s0 + plen)
                    if lo >= hi:
                        continue
                    for c in range(NCH):
                        nc.tensor.matmul(
                            pts[st][96:97, lo - s_lo : hi - s_lo],
                            amats[c][:, B - 1 : B],
                            ptiles[j][:, (c * plen + lo - s0) : (c * plen + hi - s0)],
                            start=(c == 0),
                            stop=(c == NCH - 1),
                            tile_position=(0, 96),
                        )

            # s-tile-outer: each s-tile's PSUM bank completes after its 4
            # batches (8 matmuls), so its drain overlaps later matmuls.
            for st in range(NST):
                last_st = last_g and st == NST - 1
                for bi in range(BG):
                    b = g * BG + bi
                    if last_g and bi == BG - 1:
                        mm_b15(st)
                        continue
                    for c in range(NCH):
                        nc.tensor.matmul(
                            pts[st][32 * bi : 32 * bi + 1, :],
                            amats[c][:, b : b + 1],
                            rhs_slice(b, st, c),
                            start=(c == 0),
                            stop=(c == NCH - 1),
                            tile_position=(0, 32 * bi),
                        )
                dst = outg[:, st * ST : (st + 1) * ST]
                if last_st:
                    # Final s-tile drains on DVE: it wakes ~200 ns after the
                    # final matmul's semaphore vs ~700 ns for ACT (measured),
                    # which beats column-splitting across both engines.
                    nc.vector.tensor_copy(dst, pts[st][:])
                elif st % 2 == 0:
                    nc.vector.tensor_copy(dst, pts[st][:])
                else:
                    nc.scalar.copy(dst, pts[st][:])
                if last_g and st == NST - 2:
                    # st0-2 drained: ship [4, 1536] now so the final DMA after
                    # the last piece is only [4, 512]. This rides the SYNC
                    # queue: by issue time (~55 us) the enc stream is done (its
                    # descriptors all precede this in the queue), and keeping
                    # it off ACT lets the st3 drain start there immediately.
                    nc.sync.dma_start(
                        out_r[g][:, : (NST - 1) * ST], outg_r[:, : (NST - 1) * ST]
                    )
            if last_g:
                nc.scalar.dma_start(
                    out_r[g][:, (NST - 1) * ST :], outg_r[:, (NST - 1) * ST :]
                )
            else:
                # Ship the group's [4, S] rows (batch bi at partition 32*bi)
                # on the Act queue; enc keeps streaming on the sync queue.
                nc.scalar.dma_start(out_r[g], outg_r)

    nc.compile()
    return nc


def _prep_inputs(encoder_outputs, state, W, b):
    """Build the 8 per-core input maps (heavy layout work on host)."""
    f16 = np.float16
    in_maps = []
    # [S, B, D] -> [B, D, S] once
    encT = np.ascontiguousarray(encoder_outputs.transpose(1, 2, 0))
    spk = np.ascontiguousarray(
        state.T.reshape(16, 128, B).transpose(1, 0, 2).reshape(128, 16 * B)
    ).astype(f16)
    for k in range(NCORES):
        d0 = k * DK
        e = encT[:, d0 : d0 + DK, :]  # [B, DK, S]
        # -> [c, p, B, S]
        e = np.ascontiguousarray(e.reshape(B, NCH, 128, S).transpose(1, 2, 0, 3))
        enc_k = np.ascontiguousarray(e.astype(f16).reshape(NCH, 128, B * S))
        wp = np.ascontiguousarray(
            W[d0 : d0 + DK, :].T.reshape(16, 128, DK).transpose(1, 0, 2).reshape(128, 16 * DK)
        ).astype(f16)
        bk = np.ascontiguousarray(b[d0 : d0 + DK].reshape(NCH, 128).T)
        in_maps.append({"enc": enc_k, "wp": wp, "sp": spk, "bk": bk})
    return in_maps


def kernel(encoder_outputs, state, W, b):
    global LAST_RESULTS
    mode = MODE
    if mode not in _CACHE:
        _CACHE[mode] = _build(mode)
    nc = _CACHE[mode]
    in_maps = _prep_inputs(
        np.asarray(encoder_outputs, dtype=np.float32),
        np.asarray(state, dtype=np.float32),
        np.asarray(W, dtype=np.float32),
        np.asarray(b, dtype=np.float32),
    )
    res = run_bass_kernel_spmd(nc, in_maps, core_ids=list(range(NCORES)))
    LAST_RESULTS = res
    acc = np.zeros((B, S), dtype=np.float64)
    for k in range(NCORES):
        acc += res.results[k]["out"].astype(np.float64)
    return acc.astype(np.float32)
